# revision 38
# baseline (speedup 1.0000x reference)
"""Trainium2 Bass kernel: 8192x8192 valid 3x3 cross-correlation + scalar bias.

Sharding (balanced, host-side, no collectives): 8190 output rows = 65
strips of 126. Each core gets 8 full-width strips (1008 rows, 2-row input
halo) plus a 1024-col slice of the 65th strip, so all 8 cores stream the
same 390 matmuls - no core pays a full-width ragged tail.

Compute: per strip the conv runs on TensorE as 3 PSUM-accumulating matmuls
per 512-col chunk: stationary = banded matrix A_dj[i, io] = w[i-io, dj]
(built on host), moving = the bf16 X row-strip shifted by dj columns.
PE streams 390 x 512 cols ~ 200k cycles ~ 94 us at the ~2.13 GHz effective
rate this part sustains - measured to be the structural floor for a banded
3x3 (<= 3*M useful MACs/cycle; fp8 DoubleRow would halve it but e4m3
anywhere fails the 2e-2 gate: x->6.3e-2, w->2.9e-2).

Precision vs traffic (gate 2e-2, inputs deterministic): input is host-cast
to bf16 (x-quant error 6.0e-3); output is evicted as *scaled int8*
(q = round(psum*s + bias*s), s = 127/(sum|w|*max|x|) - a rigorous
no-saturation bound; uniform quantization suits the abs-vs-global-max
metric where fp8's relative steps fail). Host divides by s and upcasts.
Total rel err 1.244e-2, bit-matching the host model (device rounds RNE).
Per-core HBM traffic: 16.8 MB bf16 loads + 8.4 MB int8 stores = 25.2 MB
~ 80 us at the ~318 GB/s/core achieved rate, comfortably under PE.

Schedule: loads on the sync HWDGE ring, stores on scalar; x/y strips
triple-buffered; PSUM evicted per 2-bank group, 5/8 on DVE tensor_scalar
(mult+add fuses scale+bias) and 3/8 on ACT activation(Identity, scale,
bias); 14 dependency-free warm-up matmuls at t=0 un-throttle the HAM
clock gate while the first strip loads; the tiny tail job (262 KB load,
6 matmuls) runs FIRST so real PE work starts ~1 us in; the first strip's
load is split 8x for the same reason; the last strip's store is split 4x
to shorten the drain. Measured ~98-100 us/iter vs the 207 us f32
baseline (PE-bound: loads 53 us, stores 26 us, evictions ~42 us all
hidden under the matmul stream).
"""

from contextlib import ExitStack

import numpy as np

import concourse.bass as bass
import concourse.tile as tile
from concourse import bacc, mybir
from concourse.bass_utils import run_bass_kernel_spmd

N_CORES = 8
H = W = 8192
KH = KW = 3
OH, OW = H - KH + 1, W - KW + 1          # 8190, 8190
TILE_M = 126                             # output rows per PE tile (K = TILE_M + 2 = 128)
CHUNK = 512                              # PSUM bank = 512 fp32
KBAND = 128

# Balanced sharding: 8190 output rows = 65 strips of 126. Each core takes 8
# full-width strips (1008 rows) plus a 1024-col slice of the 65th strip, so
# every core streams the same 390 matmuls (no full-width 16-row tail).
MAIN_ROWS = 8 * TILE_M                   # 1008 output rows per core
MAIN_IN = MAIN_ROWS + KH - 1             # 1010 input rows per core
TAIL_R0 = N_CORES * MAIN_ROWS            # 8064: first tail output row
TAIL_COLS = 1024                         # tail output cols per core
TAIL_IN_COLS = TAIL_COLS + KW - 1        # 1026

_cached = {}


CFG = dict(
    io="bf16",      # "bf16" | "f32": DRAM input dtype (bf16 halves HBM traffic)
    odt="i8",       # "i8" | "io": output dtype; i8 = scaled int8 (halves
                    # store traffic; scale = 127 / (sum|w| max|x|), rigorous)
    xbufs=4,        # input-strip pool buffers
    ybufs=3,        # output-strip pool buffers
    psbufs=8,       # PSUM pool buffers (psbufs x evw banks = all 8 banks)
    load_eng="sync",    # HWDGE ring(s) for loads (comma list round-robins)
    store_eng="scalar",  # HWDGE ring(s) for stores
    split=2,        # loads split into N column chunks (finer PE/load overlap)
    split_store=2,  # stores split into N column chunks
    last_store=4,   # final strip's store split (fires earlier -> short drain)
    evict="both38",  # "dve" | "act" | "both" | "both38": eviction engine mix
    evw=1,          # chunks per eviction group (1 = per-bank PSUM recycling)
    order="dj",     # matmul order in a group: "dj"-major shares stationary
                    # across consecutive matmuls (fewer weight reloads);
                    # "chunk"-major rotates it every matmul
    first_split=8,  # first strip's load split into N pieces (cold-start ramp)
    tail_first=1,   # schedule the tail job before the main strips
    warm_mms=14,    # dummy matmuls at t=0 (no data dep) to warm the HAM
                    # clock gate while the first strip loads
    chunkw=512,     # matmul moving width (psum cols per chunk)
    kw_used=3,      # diagnostic: matmuls per chunk (3 = correct)
    align_probe=0,  # diagnostic: drop dj column shifts (aligned reads, wrong)
    skip_compute=0,  # diagnostic: no matmuls/DVE (wrong output)
    skip_evict=0,    # diagnostic: matmuls but no eviction (wrong output)
    skip_store=0,    # diagnostic: no output stores (wrong output)
)


def _build_program(reps=1, hwreps=1, **overrides):
    cfg = {**CFG, **overrides}
    key = ("nc", reps, hwreps, tuple(sorted(cfg.items())))
    if key in _cached:
        return _cached[key]

    f32 = mybir.dt.float32
    f32r = mybir.dt.float32r
    bf16 = mybir.dt.bfloat16
    iobf = cfg["io"] == "bf16"
    xdt = bf16 if iobf else f32r         # DRAM/SBUF dtype of x strips
    mmdt = bf16 if iobf else f32r        # matmul operand dtype
    i8out = cfg["odt"] == "i8"
    ydt = mybir.dt.int8 if i8out else (bf16 if iobf else f32)

    nc = bacc.Bacc("TRN2", target_bir_lowering=False, debug=False,
                   num_devices=N_CORES)
    x_d = nc.dram_tensor("x", [MAIN_IN, W], xdt, kind="ExternalInput")
    xt_d = nc.dram_tensor("xt", [KBAND, TAIL_IN_COLS], xdt, kind="ExternalInput")
    a_d = nc.dram_tensor("a", [KBAND, KW, TILE_M], mmdt, kind="ExternalInput")
    b_d = nc.dram_tensor("b", [KBAND, 1], f32, kind="ExternalInput")
    sc_d = nc.dram_tensor("sc", [KBAND, 1], f32, kind="ExternalInput")
    y_d = nc.dram_tensor("y", [MAIN_ROWS, OW], ydt, kind="ExternalOutput")
    yt_d = nc.dram_tensor("yt", [TILE_M, TAIL_COLS], ydt, kind="ExternalOutput")

    # strip schedule: (out_row0, M) - 8 full-width tiles of 126 rows
    strips = [(r, TILE_M) for r in range(0, MAIN_ROWS, TILE_M)]

    chunk = cfg["chunkw"]
    n_chunks = (OW + chunk - 1) // chunk  # 16 (last = 510) at chunkw=512

    with tile.TileContext(nc) as tc, ExitStack() as ctx:
        const_pool = ctx.enter_context(tc.tile_pool(name="const", bufs=1))
        xpool = ctx.enter_context(tc.tile_pool(name="xin", bufs=cfg["xbufs"]))
        ypool = ctx.enter_context(tc.tile_pool(name="yout", bufs=cfg["ybufs"]))
        pspool = ctx.enter_context(
            tc.tile_pool(name="psum", bufs=cfg["psbufs"],
                         space=bass.MemorySpace.PSUM))
        load_rings = [getattr(nc, e) for e in cfg["load_eng"].split(",")]
        store_rings = [getattr(nc, e) for e in cfg["store_eng"].split(",")]
        ring_idx = [0, 0]

        class _RR:
            """Round-robin DMA ring selector (cycles per dma_start call)."""
            def __init__(self, rings, slot):
                self.rings, self.slot = rings, slot

            def dma_start(self, *a, **k):
                r = self.rings[ring_idx[self.slot] % len(self.rings)]
                ring_idx[self.slot] += 1
                return r.dma_start(*a, **k)

        load_eng = _RR(load_rings, 0)
        store_eng = _RR(store_rings, 1)

        # const loads ride the store ring (idle at head) so they don't delay
        # the first x-strip load on the sync ring
        const_eng = getattr(nc, cfg.get("const_eng", "scalar"))
        a_s = const_pool.tile([KBAND, KW, TILE_M], mmdt)
        const_eng.dma_start(a_s[:], a_d.ap())
        b_s = const_pool.tile([KBAND, 1], f32)
        const_eng.dma_start(b_s[:], b_d.ap())
        sc_s = const_pool.tile([KBAND, 1], f32)
        const_eng.dma_start(sc_s[:], sc_d.ap())

        def do_chunks(m, k, xs_src, ys_dst, width=OW):
            """Output chunks for one strip: 3 matmuls each, eviction per
            group of evw chunks (one DVE op spanning evw PSUM banks)."""
            if cfg["skip_compute"]:
                return
            evw = cfg["evw"]
            kwu = cfg["kw_used"]
            nch = (width + chunk - 1) // chunk
            for g in range(0, nch, evw):
                gchunks = range(g, min(g + evw, nch))
                gcol0 = g * chunk
                gwidth = min((g + evw) * chunk, width) - gcol0
                ps = pspool.tile([KBAND, chunk * evw], f32, tag="ps")
                if cfg["order"] == "dj":
                    mm_iter = [(c, dj) for dj in range(kwu) for c in gchunks]
                else:
                    mm_iter = [(c, dj) for c in gchunks for dj in range(kwu)]
                for c, dj in mm_iter:
                    col0 = c * chunk
                    n = min(chunk, width - col0)
                    po = col0 - gcol0
                    djx = 0 if cfg["align_probe"] else dj
                    nc.tensor.matmul(
                        ps[:m, po:po + n],
                        a_s[:k, dj, :m],
                        xs_src[:k, col0 + djx:col0 + djx + n],
                        start=(dj == 0),
                        stop=(dj == kwu - 1),
                        skip_group_check=cfg["order"] == "dj",
                    )
                if cfg["skip_evict"]:
                    continue
                gi = g // evw
                act_turn = cfg["evict"] == "act" or (
                    cfg["evict"] == "both" and gi % 3 == 2) or (
                    cfg["evict"] == "both38" and gi % 8 in (2, 5, 7))
                if act_turn:
                    nc.scalar.activation(
                        ys_dst[:m, gcol0:gcol0 + gwidth], ps[:m, :gwidth],
                        mybir.ActivationFunctionType.Identity,
                        bias=b_s[:m, :],
                        scale=sc_s[:m, :] if i8out else 1.0)
                elif i8out:
                    # out_i8 = convert(psum * s + bias*s)
                    nc.vector.tensor_scalar(
                        ys_dst[:m, gcol0:gcol0 + gwidth], ps[:m, :gwidth],
                        sc_s[:m, :], b_s[:m, :],
                        op0=mybir.AluOpType.mult, op1=mybir.AluOpType.add)
                else:
                    nc.vector.tensor_scalar_add(
                        ys_dst[:m, gcol0:gcol0 + gwidth], ps[:m, :gwidth],
                        b_s[:m, :])

        xtail_pool = ctx.enter_context(tc.tile_pool(name="xtail", bufs=2))
        ytail_pool = ctx.enter_context(tc.tile_pool(name="ytail", bufs=2))

        def emit_tail_job():
            """Tail slice: 126 rows x 1024 cols of the 65th strip. Its load
            is tiny (262 KB) so when scheduled first it primes the PE while
            the first full-width strip is still loading."""
            xst = xtail_pool.tile([KBAND, TAIL_IN_COLS], mmdt, tag="xst")
            load_eng.dma_start(xst[:], xt_d.ap())
            if cfg["skip_compute"]:
                return
            yst = ytail_pool.tile([KBAND, TAIL_COLS], ydt, tag="yst")
            do_chunks(TILE_M, KBAND, xst, yst, width=TAIL_COLS)
            if not cfg["skip_store"] and not cfg["skip_evict"]:
                store_eng.dma_start(yt_d.ap()[:, :], yst[:TILE_M, :])

        def emit_schedule():
            nsp = cfg["split"]
            for rep in range(reps):
                if rep == 0 and cfg["warm_mms"]:
                    # garbage matmuls on the (tiny, already-loading) weight
                    # tile: PE activity starts ~0.4us in, so the HAM
                    # un-throttles sooner; results are overwritten later
                    psw = pspool.tile([KBAND, chunk * cfg["evw"]], f32,
                                      tag="ps")
                    for _ in range(cfg["warm_mms"]):
                        nc.tensor.matmul(
                            psw[:TILE_M, :TILE_M], a_s[:, 0, :],
                            a_s[:, 0, :], start=True, stop=True,
                            skip_group_check=True)
                if cfg["tail_first"]:
                    emit_tail_job()
                for si, (r0, m) in enumerate(strips):
                    k = m + KH - 1
                    xs = xpool.tile([KBAND, W], mmdt, tag="xs")
                    # finer pieces for the very first load so PE starts sooner
                    nld = cfg["first_split"] if si == 0 else nsp
                    for sp in range(nld):
                        c0, c1 = W * sp // nld, W * (sp + 1) // nld
                        load_eng.dma_start(xs[:k, c0:c1],
                                           x_d.ap()[r0:r0 + k, c0:c1])
                    if si == 0 and not cfg["tail_first"]:
                        emit_tail_job()
                    if cfg["skip_compute"]:
                        continue
                    ys = ypool.tile([KBAND, OW], ydt, tag="ys")
                    do_chunks(m, k, xs, ys)
                    if not cfg["skip_store"] and not cfg["skip_evict"]:
                        nss = (cfg["last_store"] if si == len(strips) - 1
                               else cfg["split_store"])
                        for sp in range(nss):
                            c0, c1 = OW * sp // nss, OW * (sp + 1) // nss
                            store_eng.dma_start(y_d.ap()[r0:r0 + m, c0:c1],
                                                ys[:m, c0:c1])

        if hwreps > 1:
            with tc.For_i(0, hwreps):
                emit_schedule()
        else:
            emit_schedule()

    nc.compile()
    _cached[key] = nc
    return nc


def _out_scale(X, weight, io=None):
    """int8 output scale: s = 127 / (sum|w| * max|x| + |bias-free bound|).
    Rigorous bound on |conv out| -> no int8 saturation for any input.
    Computed on the quantized values the device actually multiplies."""
    io = CFG["io"] if io is None else io
    if io == "bf16":
        import ml_dtypes
        w = np.asarray(weight).astype(ml_dtypes.bfloat16).astype(np.float32)
        xmax = np.float32(
            np.abs(np.asarray(X).astype(ml_dtypes.bfloat16)
                   .astype(np.float32)).max())
    else:
        w = np.asarray(weight, dtype=np.float32)
        xmax = np.float32(np.abs(np.asarray(X, dtype=np.float32)).max())
    bound = np.float32(np.abs(w).sum()) * xmax
    return np.float32(127.0) / max(bound, np.float32(1e-30))


def _host_inputs(X, weight, bias, io=None, odt=None):
    """Build the 8 per-core input maps from full inputs."""
    io = CFG["io"] if io is None else io
    odt = CFG["odt"] if odt is None else odt
    X = np.ascontiguousarray(X, dtype=np.float32)
    weight = np.asarray(weight, dtype=np.float32)
    bias = np.asarray(bias, dtype=np.float32)

    # banded stationary matrices: a[p, dj, io] = weight[p - io, dj]
    a = np.zeros((KBAND, KW, TILE_M), dtype=np.float32)
    for di in range(KH):
        for dj in range(KW):
            for o in range(TILE_M):
                a[o + di, dj, o] = weight[di, dj]

    s = _out_scale(X, weight, io) if odt == "i8" else np.float32(1.0)
    b = np.full((KBAND, 1), bias[0] * s, dtype=np.float32)
    sc = np.full((KBAND, 1), s, dtype=np.float32)

    if io == "bf16":
        import ml_dtypes
        X = X.astype(ml_dtypes.bfloat16)
        a = a.astype(ml_dtypes.bfloat16)

    # tail strip inputs: rows [8064, 8192), cols [1024c, 1024c+1026)
    # (core 7 needs cols up to 8193; pad 2 zero cols, trimmed on unshard)
    Xtail = np.concatenate(
        [X[TAIL_R0:], np.zeros((KBAND, KW - 1), dtype=X.dtype)], axis=1)

    in_maps = []
    for c in range(N_CORES):
        r0 = c * MAIN_ROWS
        c0 = c * TAIL_COLS
        in_maps.append({
            "x": np.ascontiguousarray(X[r0:r0 + MAIN_IN]),
            "xt": np.ascontiguousarray(Xtail[:, c0:c0 + TAIL_IN_COLS]),
            "a": a,
            "b": b,
            "sc": sc,
        })
    return in_maps


def kernel(X, weight, bias):
    nc = _build_program()
    in_maps = _host_inputs(X, weight, bias)
    res = run_bass_kernel_spmd(nc, in_maps, core_ids=list(range(N_CORES)))
    inv_s = (np.float32(1.0) / _out_scale(X, weight)
             if CFG["odt"] == "i8" else np.float32(1.0))
    out = np.empty((OH, OW), dtype=np.float32)
    for c in range(N_CORES):
        out[c * MAIN_ROWS:(c + 1) * MAIN_ROWS] = np.asarray(
            res.results[c]["y"], dtype=np.float32) * inv_s
        c0 = c * TAIL_COLS
        w_valid = min(TAIL_COLS, OW - c0)
        out[TAIL_R0:, c0:c0 + w_valid] = np.asarray(
            res.results[c]["yt"], dtype=np.float32)[:, :w_valid] * inv_s
    return out


# revision 39
# speedup vs baseline: 1.1418x; 1.1418x over previous
"""Trainium2 Bass kernel: 8192x8192 valid 3x3 cross-correlation + scalar bias.

Sharding (balanced, host-side, no collectives): 8190 output rows = 65
strips of 126. Each core gets 8 full-width strips (1008 rows, 2-row input
halo) plus a 1024-col slice of the 65th strip, so all 8 cores stream the
same 390 matmuls - no core pays a full-width ragged tail.

Compute: per strip the conv runs on TensorE as 3 PSUM-accumulating matmuls
per 512-col chunk: stationary = banded matrix A_dj[i, io] = w[i-io, dj]
(built on host), moving = the bf16 X row-strip shifted by dj columns.
PE streams 390 x 512 cols ~ 200k cycles ~ 94 us at the ~2.13 GHz effective
rate this part sustains - measured to be the structural floor for a banded
3x3 (<= 3*M useful MACs/cycle; fp8 DoubleRow would halve it but e4m3
anywhere fails the 2e-2 gate: x->6.3e-2, w->2.9e-2).

Precision vs traffic (gate 2e-2, inputs deterministic): input is host-cast
to bf16 (x-quant error 6.0e-3); output is evicted as *scaled int8*
(q = round(psum*s + bias*s), s = 127/(sum|w|*max|x|) - a rigorous
no-saturation bound; uniform quantization suits the abs-vs-global-max
metric where fp8's relative steps fail). Host divides by s and upcasts.
Total rel err 1.244e-2, bit-matching the host model (device rounds RNE).
Per-core HBM traffic: 16.8 MB bf16 loads + 8.4 MB int8 stores = 25.2 MB
~ 80 us at the ~318 GB/s/core achieved rate, comfortably under PE.

Schedule: loads on the sync HWDGE ring split in 1 MB halves (finer
load->matmul dependencies smooth PE micro-stalls: -3 us vs whole-strip
loads in the fast power state), stores on scalar; x strips 4-buffered,
y strips 3-buffered; PSUM evicted per single bank (evw=1, 8 tiles - the
finest recycling granularity), 5/8 on DVE tensor_scalar (mult+add fuses
scale+bias) and 3/8 on ACT activation(Identity, scale, bias); 14
dependency-free warm-up matmuls at t=0 un-throttle the HAM clock gate
while the first strip loads; the tiny tail job (262 KB load, 6 matmuls)
runs FIRST so real PE work starts ~1 us in; the first strip's load is
split 8x for the same reason; the last strip's store is split 4x to
shorten the drain. Measured ~97.4 us/iter (fast chip state; the part
drifts to ~110-115 us in a throttled state regardless of config) vs the
207 us f32 baseline. PE-bound: loads 53 us, stores 26 us, evictions
~42 us all hidden under the matmul stream.
"""

from contextlib import ExitStack

import numpy as np

import concourse.bass as bass
import concourse.tile as tile
from concourse import bacc, mybir
from concourse.bass_utils import run_bass_kernel_spmd

N_CORES = 8
H = W = 8192
KH = KW = 3
OH, OW = H - KH + 1, W - KW + 1          # 8190, 8190
TILE_M = 126                             # output rows per PE tile (K = TILE_M + 2 = 128)
CHUNK = 512                              # PSUM bank = 512 fp32
KBAND = 128

# Balanced sharding: 8190 output rows = 65 strips of 126. Each core takes 8
# full-width strips (1008 rows) plus a 1024-col slice of the 65th strip, so
# every core streams the same 390 matmuls (no full-width 16-row tail).
MAIN_ROWS = 8 * TILE_M                   # 1008 output rows per core
MAIN_IN = MAIN_ROWS + KH - 1             # 1010 input rows per core
TAIL_R0 = N_CORES * MAIN_ROWS            # 8064: first tail output row
TAIL_COLS = 1024                         # tail output cols per core
TAIL_IN_COLS = TAIL_COLS + KW - 1        # 1026

_cached = {}


CFG = dict(
    io="bf16",      # "bf16" | "f32": DRAM input dtype (bf16 halves HBM traffic)
    odt="i8",       # "i8" | "io": output dtype; i8 = scaled int8 (halves
                    # store traffic; scale = 127 / (sum|w| max|x|), rigorous)
    xbufs=4,        # input-strip pool buffers
    ybufs=3,        # output-strip pool buffers
    psbufs=8,       # PSUM pool buffers (psbufs x evw banks = all 8 banks)
    load_eng="sync",    # HWDGE ring(s) for loads (comma list round-robins)
    store_eng="scalar",  # HWDGE ring(s) for stores
    split=2,        # loads split into N column chunks (finer PE/load overlap)
    split_store=2,  # stores split into N column chunks
    last_store=4,   # final strip's store split (fires earlier -> short drain)
    evict="both38",  # "dve" | "act" | "both" | "both38": eviction engine mix
    evw=1,          # chunks per eviction group (1 = per-bank PSUM recycling)
    order="dj",     # matmul order in a group: "dj"-major shares stationary
                    # across consecutive matmuls (fewer weight reloads);
                    # "chunk"-major rotates it every matmul
    first_split=8,  # first strip's load split into N pieces (cold-start ramp)
    tail_first=1,   # schedule the tail job before the main strips
    warm_mms=14,    # dummy matmuls at t=0 (no data dep) to warm the HAM
                    # clock gate while the first strip loads
    chunkw=512,     # matmul moving width (psum cols per chunk)
    kw_used=3,      # diagnostic: matmuls per chunk (3 = correct)
    align_probe=0,  # diagnostic: drop dj column shifts (aligned reads, wrong)
    skip_compute=0,  # diagnostic: no matmuls/DVE (wrong output)
    skip_evict=0,    # diagnostic: matmuls but no eviction (wrong output)
    skip_store=0,    # diagnostic: no output stores (wrong output)
)


def _build_program(reps=1, hwreps=1, **overrides):
    cfg = {**CFG, **overrides}
    key = ("nc", reps, hwreps, tuple(sorted(cfg.items())))
    if key in _cached:
        return _cached[key]

    f32 = mybir.dt.float32
    f32r = mybir.dt.float32r
    bf16 = mybir.dt.bfloat16
    iobf = cfg["io"] == "bf16"
    xdt = bf16 if iobf else f32r         # DRAM/SBUF dtype of x strips
    mmdt = bf16 if iobf else f32r        # matmul operand dtype
    i8out = cfg["odt"] == "i8"
    ydt = mybir.dt.int8 if i8out else (bf16 if iobf else f32)

    nc = bacc.Bacc("TRN2", target_bir_lowering=False, debug=False,
                   num_devices=N_CORES)
    x_d = nc.dram_tensor("x", [MAIN_IN, W], xdt, kind="ExternalInput")
    xt_d = nc.dram_tensor("xt", [KBAND, TAIL_IN_COLS], xdt, kind="ExternalInput")
    a_d = nc.dram_tensor("a", [KBAND, KW, TILE_M], mmdt, kind="ExternalInput")
    b_d = nc.dram_tensor("b", [KBAND, 1], f32, kind="ExternalInput")
    sc_d = nc.dram_tensor("sc", [KBAND, 1], f32, kind="ExternalInput")
    y_d = nc.dram_tensor("y", [MAIN_ROWS, OW], ydt, kind="ExternalOutput")
    yt_d = nc.dram_tensor("yt", [TILE_M, TAIL_COLS], ydt, kind="ExternalOutput")

    # strip schedule: (out_row0, M) - 8 full-width tiles of 126 rows
    strips = [(r, TILE_M) for r in range(0, MAIN_ROWS, TILE_M)]

    chunk = cfg["chunkw"]
    n_chunks = (OW + chunk - 1) // chunk  # 16 (last = 510) at chunkw=512

    with tile.TileContext(nc) as tc, ExitStack() as ctx:
        const_pool = ctx.enter_context(tc.tile_pool(name="const", bufs=1))
        xpool = ctx.enter_context(tc.tile_pool(name="xin", bufs=cfg["xbufs"]))
        ypool = ctx.enter_context(tc.tile_pool(name="yout", bufs=cfg["ybufs"]))
        pspool = ctx.enter_context(
            tc.tile_pool(name="psum", bufs=cfg["psbufs"],
                         space=bass.MemorySpace.PSUM))
        load_rings = [getattr(nc, e) for e in cfg["load_eng"].split(",")]
        store_rings = [getattr(nc, e) for e in cfg["store_eng"].split(",")]
        ring_idx = [0, 0]

        class _RR:
            """Round-robin DMA ring selector (cycles per dma_start call)."""
            def __init__(self, rings, slot):
                self.rings, self.slot = rings, slot

            def dma_start(self, *a, **k):
                r = self.rings[ring_idx[self.slot] % len(self.rings)]
                ring_idx[self.slot] += 1
                return r.dma_start(*a, **k)

        load_eng = _RR(load_rings, 0)
        store_eng = _RR(store_rings, 1)

        # const loads ride the store ring (idle at head) so they don't delay
        # the first x-strip load on the sync ring
        const_eng = getattr(nc, cfg.get("const_eng", "scalar"))
        a_s = const_pool.tile([KBAND, KW, TILE_M], mmdt)
        const_eng.dma_start(a_s[:], a_d.ap())
        b_s = const_pool.tile([KBAND, 1], f32)
        const_eng.dma_start(b_s[:], b_d.ap())
        sc_s = const_pool.tile([KBAND, 1], f32)
        const_eng.dma_start(sc_s[:], sc_d.ap())

        def do_chunks(m, k, xs_src, ys_dst, width=OW):
            """Output chunks for one strip: 3 matmuls each, eviction per
            group of evw chunks (one DVE op spanning evw PSUM banks)."""
            if cfg["skip_compute"]:
                return
            evw = cfg["evw"]
            kwu = cfg["kw_used"]
            nch = (width + chunk - 1) // chunk
            for g in range(0, nch, evw):
                gchunks = range(g, min(g + evw, nch))
                gcol0 = g * chunk
                gwidth = min((g + evw) * chunk, width) - gcol0
                ps = pspool.tile([KBAND, chunk * evw], f32, tag="ps")
                if cfg["order"] == "dj":
                    mm_iter = [(c, dj) for dj in range(kwu) for c in gchunks]
                else:
                    mm_iter = [(c, dj) for c in gchunks for dj in range(kwu)]
                for c, dj in mm_iter:
                    col0 = c * chunk
                    n = min(chunk, width - col0)
                    po = col0 - gcol0
                    djx = 0 if cfg["align_probe"] else dj
                    nc.tensor.matmul(
                        ps[:m, po:po + n],
                        a_s[:k, dj, :m],
                        xs_src[:k, col0 + djx:col0 + djx + n],
                        start=(dj == 0),
                        stop=(dj == kwu - 1),
                        skip_group_check=cfg["order"] == "dj",
                    )
                if cfg["skip_evict"]:
                    continue
                gi = g // evw
                act_turn = cfg["evict"] == "act" or (
                    cfg["evict"] == "both" and gi % 3 == 2) or (
                    cfg["evict"] == "both38" and gi % 8 in (2, 5, 7))
                if act_turn:
                    nc.scalar.activation(
                        ys_dst[:m, gcol0:gcol0 + gwidth], ps[:m, :gwidth],
                        mybir.ActivationFunctionType.Identity,
                        bias=b_s[:m, :],
                        scale=sc_s[:m, :] if i8out else 1.0)
                elif i8out:
                    # out_i8 = convert(psum * s + bias*s)
                    nc.vector.tensor_scalar(
                        ys_dst[:m, gcol0:gcol0 + gwidth], ps[:m, :gwidth],
                        sc_s[:m, :], b_s[:m, :],
                        op0=mybir.AluOpType.mult, op1=mybir.AluOpType.add)
                else:
                    nc.vector.tensor_scalar_add(
                        ys_dst[:m, gcol0:gcol0 + gwidth], ps[:m, :gwidth],
                        b_s[:m, :])

        xtail_pool = ctx.enter_context(tc.tile_pool(name="xtail", bufs=2))
        ytail_pool = ctx.enter_context(tc.tile_pool(name="ytail", bufs=2))

        def emit_tail_job():
            """Tail slice: 126 rows x 1024 cols of the 65th strip. Its load
            is tiny (262 KB) so when scheduled first it primes the PE while
            the first full-width strip is still loading."""
            xst = xtail_pool.tile([KBAND, TAIL_IN_COLS], mmdt, tag="xst")
            load_eng.dma_start(xst[:], xt_d.ap())
            if cfg["skip_compute"]:
                return
            yst = ytail_pool.tile([KBAND, TAIL_COLS], ydt, tag="yst")
            do_chunks(TILE_M, KBAND, xst, yst, width=TAIL_COLS)
            if not cfg["skip_store"] and not cfg["skip_evict"]:
                store_eng.dma_start(yt_d.ap()[:, :], yst[:TILE_M, :])

        def emit_schedule():
            nsp = cfg["split"]
            for rep in range(reps):
                if rep == 0 and cfg["warm_mms"]:
                    # garbage matmuls on the (tiny, already-loading) weight
                    # tile: PE activity starts ~0.4us in, so the HAM
                    # un-throttles sooner; results are overwritten later
                    psw = pspool.tile([KBAND, chunk * cfg["evw"]], f32,
                                      tag="ps")
                    for _ in range(cfg["warm_mms"]):
                        nc.tensor.matmul(
                            psw[:TILE_M, :TILE_M], a_s[:, 0, :],
                            a_s[:, 0, :], start=True, stop=True,
                            skip_group_check=True)
                if cfg["tail_first"]:
                    emit_tail_job()
                for si, (r0, m) in enumerate(strips):
                    k = m + KH - 1
                    xs = xpool.tile([KBAND, W], mmdt, tag="xs")
                    # finer pieces for the very first load so PE starts sooner
                    nld = cfg["first_split"] if si == 0 else nsp
                    for sp in range(nld):
                        c0, c1 = W * sp // nld, W * (sp + 1) // nld
                        load_eng.dma_start(xs[:k, c0:c1],
                                           x_d.ap()[r0:r0 + k, c0:c1])
                    if si == 0 and not cfg["tail_first"]:
                        emit_tail_job()
                    if cfg["skip_compute"]:
                        continue
                    ys = ypool.tile([KBAND, OW], ydt, tag="ys")
                    do_chunks(m, k, xs, ys)
                    if not cfg["skip_store"] and not cfg["skip_evict"]:
                        nss = (cfg["last_store"] if si == len(strips) - 1
                               else cfg["split_store"])
                        for sp in range(nss):
                            c0, c1 = OW * sp // nss, OW * (sp + 1) // nss
                            store_eng.dma_start(y_d.ap()[r0:r0 + m, c0:c1],
                                                ys[:m, c0:c1])

        if hwreps > 1:
            with tc.For_i(0, hwreps):
                emit_schedule()
        else:
            emit_schedule()

    nc.compile()
    _cached[key] = nc
    return nc


def _out_scale(X, weight, io=None):
    """int8 output scale: s = 127 / (sum|w| * max|x| + |bias-free bound|).
    Rigorous bound on |conv out| -> no int8 saturation for any input.
    Computed on the quantized values the device actually multiplies."""
    io = CFG["io"] if io is None else io
    if io == "bf16":
        import ml_dtypes
        w = np.asarray(weight).astype(ml_dtypes.bfloat16).astype(np.float32)
        xmax = np.float32(
            np.abs(np.asarray(X).astype(ml_dtypes.bfloat16)
                   .astype(np.float32)).max())
    else:
        w = np.asarray(weight, dtype=np.float32)
        xmax = np.float32(np.abs(np.asarray(X, dtype=np.float32)).max())
    bound = np.float32(np.abs(w).sum()) * xmax
    return np.float32(127.0) / max(bound, np.float32(1e-30))


def _host_inputs(X, weight, bias, io=None, odt=None):
    """Build the 8 per-core input maps from full inputs."""
    io = CFG["io"] if io is None else io
    odt = CFG["odt"] if odt is None else odt
    X = np.ascontiguousarray(X, dtype=np.float32)
    weight = np.asarray(weight, dtype=np.float32)
    bias = np.asarray(bias, dtype=np.float32)

    # banded stationary matrices: a[p, dj, io] = weight[p - io, dj]
    a = np.zeros((KBAND, KW, TILE_M), dtype=np.float32)
    for di in range(KH):
        for dj in range(KW):
            for o in range(TILE_M):
                a[o + di, dj, o] = weight[di, dj]

    s = _out_scale(X, weight, io) if odt == "i8" else np.float32(1.0)
    b = np.full((KBAND, 1), bias[0] * s, dtype=np.float32)
    sc = np.full((KBAND, 1), s, dtype=np.float32)

    if io == "bf16":
        import ml_dtypes
        X = X.astype(ml_dtypes.bfloat16)
        a = a.astype(ml_dtypes.bfloat16)

    # tail strip inputs: rows [8064, 8192), cols [1024c, 1024c+1026)
    # (core 7 needs cols up to 8193; pad 2 zero cols, trimmed on unshard)
    Xtail = np.concatenate(
        [X[TAIL_R0:], np.zeros((KBAND, KW - 1), dtype=X.dtype)], axis=1)

    in_maps = []
    for c in range(N_CORES):
        r0 = c * MAIN_ROWS
        c0 = c * TAIL_COLS
        in_maps.append({
            "x": np.ascontiguousarray(X[r0:r0 + MAIN_IN]),
            "xt": np.ascontiguousarray(Xtail[:, c0:c0 + TAIL_IN_COLS]),
            "a": a,
            "b": b,
            "sc": sc,
        })
    return in_maps


def kernel(X, weight, bias):
    nc = _build_program()
    in_maps = _host_inputs(X, weight, bias)
    res = run_bass_kernel_spmd(nc, in_maps, core_ids=list(range(N_CORES)))
    inv_s = (np.float32(1.0) / _out_scale(X, weight)
             if CFG["odt"] == "i8" else np.float32(1.0))
    out = np.empty((OH, OW), dtype=np.float32)
    for c in range(N_CORES):
        out[c * MAIN_ROWS:(c + 1) * MAIN_ROWS] = np.asarray(
            res.results[c]["y"], dtype=np.float32) * inv_s
        c0 = c * TAIL_COLS
        w_valid = min(TAIL_COLS, OW - c0)
        out[TAIL_R0:, c0:c0 + w_valid] = np.asarray(
            res.results[c]["yt"], dtype=np.float32)[:, :w_valid] * inv_s
    return out
